# revision 28
# baseline (speedup 1.0000x reference)
# Local (sliding-window, strictly-causal) multi-head attention for Trainium2.
#
# Problem: nn_LocalAttention  (B=2, S=4096, MD=AD=1024, NH=8, HD=128, window=256)
#   q = query @ Wq.T ; per-head scores q.k/sqrt(HD) masked to col in [row-256, row-1];
#   softmax; out = w @ v ; rows with no valid keys zeroed; out @ Wo.T.
#
# Sharding (8 cores): batch (2) x sequence chunks (4 x 1024 rows).  Each core runs
# the whole pipeline for its 1024 query rows using a 256-row K/V halo, so the 8
# output shards are disjoint and the gather is pure concatenation.  Weights are
# replicated.
#
# Device pipeline (v2):
#   - All wide matmuls (out free-dim >= 256) run as float32r: 1 cycle/row on the
#     PE array instead of fp32's 4.
#   - Scores are computed key-block-major: for each of the 10 halo key blocks,
#     one wide matmul produces scoresT[k, q] for the (up to) 3 query tiles that
#     attend to that block, on top of a mask bias preloaded into PSUM by a bf16
#     identity matmul.  The mask band is shift-invariant, so a single 384-column
#     interior bias tile is shared by key blocks 2..9 (edge blocks get their own
#     small tiles; the s0==0 halo-padding cases are baked into those per-core).
#   - exp() (no max subtraction: scores are O(1), masked entries -1e5 -> exp==0)
#     is written as bf16; the PV matmul and the 128x128 output transpose run in
#     bf16 (1 cycle/row).  V carries an interleaved ones column per head so the
#     softmax denominator falls out of the PV matmul; normalization is a
#     per-partition scalar multiply.
#   - The Wo projection accumulates all 8 heads into PSUM (f32r, 512-wide) and
#     streams out row-contiguous.

import math

import numpy as np

try:  # numpy bf16 via ml_dtypes (jax dependency, always present here)
    import ml_dtypes

    BF16_NP = np.dtype(ml_dtypes.bfloat16)
except ImportError:  # pragma: no cover
    BF16_NP = None

import concourse.bass as bass
import concourse.tile as tile
from concourse import bacc, mybir
from concourse.bass_utils import run_bass_kernel_spmd
from concourse.masks import make_identity

F32 = mybir.dt.float32
F32R = mybir.dt.float32r  # fast fp32 matmul mode: 1 cycle/row when out width >= 256
BF16 = mybir.dt.bfloat16

NH = 8       # heads
HD = 128     # head dim
B = 2        # batch
S = 4096     # sequence
MD = 1024    # model dim
AD = 1024    # attn dim
WIN = 256    # window
C = 1024     # query rows per core (chunk)
NQT = C // 128          # 8 query tiles per chunk
HALO = WIN + C          # 1280 key/value rows per core
NKB = HALO // 128       # 10 key blocks
VROW = NH * (HD + 1)    # 1032: v with a ones column interleaved per head
NCORES = 8
MASK_NEG = -1.0e5       # exp(-1e5 + O(1)) == 0 exactly in f32/bf16
EXP = mybir.ActivationFunctionType.Exp


# ----------------------------------------------------------------------------
# device program
# ----------------------------------------------------------------------------

def _emit(ctx, tc: tile.TileContext, qcT, wqT, woT, kT, vp, biasT, out):
    nc = tc.nc

    const_pool = ctx.enter_context(tc.tile_pool(name="const", bufs=1))
    ident = const_pool.tile([128, 128], BF16)
    make_identity(nc, ident)

    # pools that live for the whole kernel
    kT_pool = ctx.enter_context(tc.tile_pool(name="kT", bufs=1))
    bias_pool = ctx.enter_context(tc.tile_pool(name="bias", bufs=1))
    qT_pool = ctx.enter_context(tc.tile_pool(name="qT", bufs=1))

    kT_sb = kT_pool.tile([128, NH, HALO], BF16)
    # bias columns: [0:128) kb==0 | [128:384) kb==1 | [384:768) interior kb>=2
    bias_sb = bias_pool.tile([128, 768], BF16)
    qT_sb = qT_pool.tile([128, NH, C], BF16)

    # ---------------- phase 1: q projection -> qT[d, h, t] -------------------
    # mt-outer over head pairs so the PE consumes qc/wq chunks as they stream
    # in; two [128, 2, 1024] PSUM tiles (4 banks each) ping-pong.
    with tc.tile_pool(name="qc", bufs=1) as qc_pool, \
         tc.tile_pool(name="wq", bufs=1) as wq_pool, \
         tc.tile_pool(name="qp_psum", bufs=1, space="PSUM") as qp_psum:
        qc_sb = qc_pool.tile([128, 8, C], BF16)
        wq_sb = wq_pool.tile([128, 8, AD], BF16)
        # qc/wq chunks first (they gate everything).  The first head-quad only
        # needs wq columns 0:512, so those halves stream first; kT/bias/second
        # wq halves follow for the attention phase.
        for mt in range(8):
            nc.sync.dma_start(out=qc_sb[:, mt, :], in_=qcT[mt * 128:(mt + 1) * 128, :])
            nc.sync.dma_start(out=wq_sb[:, mt, 0:512],
                              in_=wqT[mt * 128:(mt + 1) * 128, 0:512])
        for mt in range(8):
            nc.sync.dma_start(out=wq_sb[:, mt, 512:1024],
                              in_=wqT[mt * 128:(mt + 1) * 128, 512:1024])
        for h in range(NH):
            nc.sync.dma_start(out=kT_sb[:, h, :], in_=kT[h])
        nc.sync.dma_start(out=bias_sb, in_=biasT)

        for gpair in range(2):          # (groups 0,1) then (groups 2,3)
            ps0 = qp_psum.tile([128, 2, C], F32)
            ps1 = qp_psum.tile([128, 2, C], F32)
            for mt in range(8):
                for g, ps in ((2 * gpair, ps0), (2 * gpair + 1, ps1)):
                    for hh in range(2):
                        h = 2 * g + hh
                        lhsT = wq_sb[:, mt, h * 128:(h + 1) * 128]
                        for nn in range(2):
                            nc.tensor.matmul(
                                ps[:, hh, nn * 512:(nn + 1) * 512],
                                lhsT=lhsT,
                                rhs=qc_sb[:, mt, nn * 512:(nn + 1) * 512],
                                start=(mt == 0),
                                stop=(mt == 7),
                            )
            for g, ps in ((2 * gpair, ps0), (2 * gpair + 1, ps1)):
                for hh in range(2):   # per-head copies split across ACT and DVE
                    dst = qT_sb[:, 2 * g + hh, :]
                    if hh == 0:
                        nc.scalar.copy(dst, ps[:, hh, :])
                    else:
                        nc.vector.tensor_copy(dst, ps[:, hh, :])

    # ---------------- phase 2: attention ------------------------------------
    with tc.tile_pool(name="vp", bufs=1) as vp_pool, \
         tc.tile_pool(name="wo", bufs=1) as wo_pool, \
         tc.tile_pool(name="outT", bufs=1) as outT_pool:

        vp_sb = vp_pool.tile([128, NKB, VROW], BF16)
        for blk in range(NKB):
            nc.sync.dma_start(out=vp_sb[:, blk, :], in_=vp[blk])
        wo_sb = wo_pool.tile([128, NH, MD], F32R)
        nc.sync.dma_start(out=wo_sb, in_=woT.rearrange("(h d) o -> d h o", d=128))
        outT_sb = outT_pool.tile([128, NH, NQT, 128], F32R)

        with tc.tile_pool(name="e", bufs=2) as e_pool, \
             tc.tile_pool(name="oh", bufs=3) as oh_pool, \
             tc.tile_pool(name="r", bufs=3) as r_pool, \
             tc.tile_pool(name="sc_psum", bufs=2, space="PSUM") as sc_psum, \
             tc.tile_pool(name="ov_psum", bufs=2, space="PSUM") as ov_psum, \
             tc.tile_pool(name="tr_psum", bufs=2, space="PSUM") as tr_psum:

            for h in range(NH):
                e_sb = e_pool.tile([128, NKB, 384], BF16)
                for kb_a in range(0, NKB, 2):       # key-block pairs share a PSUM tile
                    # 512-wide slots: each matmul output must stay in one bank
                    s_ps = sc_psum.tile([128, 2, 512], F32)
                    ws = []
                    for p in range(2):
                        kb = kb_a + p
                        qlo = max(0, kb - 2)
                        qhi = min(NQT - 1, kb)
                        w = (qhi - qlo + 1) * 128
                        ws.append(w)
                        if kb == 0:
                            bsl = bias_sb[:, 0:128]
                        elif kb == 1:
                            bsl = bias_sb[:, 128:384]
                        else:
                            bsl = bias_sb[:, 384:384 + w]
                        nc.tensor.matmul(
                            s_ps[:, p, 0:w], lhsT=ident, rhs=bsl,
                            start=True, stop=False)
                        nc.tensor.matmul(
                            s_ps[:, p, 0:w],
                            lhsT=kT_sb[:, h, kb * 128:(kb + 1) * 128],
                            rhs=qT_sb[:, h, qlo * 128:(qhi + 1) * 128],
                            start=False,
                            stop=True,
                        )
                    if ws[0] == 384 and ws[1] == 384:   # interior pair: fused exp
                        nc.scalar.activation(
                            e_sb[:, kb_a:kb_a + 2, :],
                            s_ps[:, :, 0:384],
                            EXP,
                        )
                    else:
                        for p in range(2):
                            nc.scalar.activation(
                                e_sb[:, kb_a + p, 0:ws[p]], s_ps[:, p, 0:ws[p]], EXP)

                for qp in range(NQT // 2):          # query-tile pairs
                    o_ps = ov_psum.tile([128, 2, HD + 1], F32)
                    for j in range(2):
                        qt = 2 * qp + j
                        for sub in range(3):
                            kb = qt + sub
                            off = (qt - max(0, kb - 2)) * 128
                            nc.tensor.matmul(
                                o_ps[:, j, :],
                                lhsT=e_sb[:, kb, off:off + 128],
                                rhs=vp_sb[:, kb, h * (HD + 1):(h + 1) * (HD + 1)],
                                start=(sub == 0),
                                stop=(sub == 2),
                            )
                    r_sb = r_pool.tile([128, 2], F32)
                    nc.vector.reciprocal(
                        r_sb, o_ps[:, :, HD:HD + 1].rearrange("p a b -> p (a b)"))
                    t_ps = tr_psum.tile([128, 2, 128], BF16)
                    for j in range(2):
                        oh_sb = oh_pool.tile([128, 128], BF16)
                        nc.vector.tensor_scalar_mul(
                            oh_sb, o_ps[:, j, 0:HD], r_sb[:, j:j + 1])
                        nc.tensor.transpose(t_ps[:, j, :], oh_sb, ident)
                    dst = outT_sb[:, h, 2 * qp:2 * qp + 2, :].rearrange(
                        "p a b -> p (a b)")
                    src = t_ps.rearrange("p a b -> p (a b)")
                    if qp % 2 == 0:
                        nc.vector.tensor_copy(dst, src)
                    else:
                        nc.scalar.copy(dst, src)

        # ---------------- phase 3: output projection -------------------------
        with tc.tile_pool(name="stage", bufs=2) as stage_pool, \
             tc.tile_pool(name="fi_psum", bufs=2, space="PSUM") as fi_psum:
            for qt in range(NQT):
                f_ps = fi_psum.tile([128, MD], F32)
                for h in range(NH):
                    lhsT = outT_sb[:, h, qt, :]
                    for nn in range(2):
                        nc.tensor.matmul(
                            f_ps[:, nn * 512:(nn + 1) * 512],
                            lhsT=lhsT,
                            rhs=wo_sb[:, h, nn * 512:(nn + 1) * 512],
                            start=(h == 0),
                            stop=(h == NH - 1),
                        )
                st = stage_pool.tile([128, MD], F32)
                nc.scalar.copy(st[:, 0:512], f_ps[:, 0:512])
                nc.vector.tensor_copy(st[:, 512:1024], f_ps[:, 512:1024])
                nc.sync.dma_start(
                    out=out[qt * 128:(qt + 1) * 128, 0:512], in_=st[:, 0:512])
                nc.sync.dma_start(
                    out=out[qt * 128:(qt + 1) * 128, 512:1024], in_=st[:, 512:1024])


_CACHED_NC = {}


def _build_program(iters: int = 1):
    if iters in _CACHED_NC:
        return _CACHED_NC[iters]
    nc = bacc.Bacc("TRN2", target_bir_lowering=False, debug=False)
    qcT = nc.dram_tensor("qcT", [MD, C], BF16, kind="ExternalInput").ap()
    wqT = nc.dram_tensor("wqT", [MD, AD], BF16, kind="ExternalInput").ap()
    woT = nc.dram_tensor("woT", [AD, MD], F32R, kind="ExternalInput").ap()
    kT = nc.dram_tensor("kT", [NH, HD, HALO], BF16, kind="ExternalInput").ap()
    vp = nc.dram_tensor("vp", [NKB, 128, VROW], BF16, kind="ExternalInput").ap()
    biasT = nc.dram_tensor("biasT", [128, 768], BF16, kind="ExternalInput").ap()
    out = nc.dram_tensor("out", [C, MD], F32, kind="ExternalOutput").ap()
    from contextlib import ExitStack

    with tile.TileContext(nc) as tc:
        for _ in range(iters):
            with ExitStack() as ctx:
                _emit(ctx, tc, qcT, wqT, woT, kT, vp, biasT, out)
    nc.compile()
    _CACHED_NC[iters] = nc
    return nc


# ----------------------------------------------------------------------------
# host-side shard construction
# ----------------------------------------------------------------------------

def _build_bias(s0: int) -> np.ndarray:
    """Mask bias, bf16, columns [kb0 | kb1 | interior]: [128, 768].

    interior[k, j] (j = p*128 + c over the 3 query tiles kb-2..kb of any
    interior key block): valid iff 1 <= j - k <= WIN.
    kb==0 stores query tile 0 only (j offset 256 of the interior pattern);
    kb==1 stores query tiles 0..1 (j offset 128).  For the s0==0 core those
    two blocks sit in the zero-padded halo: fully masked, except element
    [0, 0] which gives query row 0 one unmasked zero-valued key so its
    softmax output is exactly 0 (matching the reference's has_valid zeroing).
    """
    kk = np.arange(128)[:, None]
    jj = np.arange(384)[None, :]
    interior = np.where((jj - kk >= 1) & (jj - kk <= WIN), 0.0, MASK_NEG)

    b = np.empty((128, 768), np.float32)
    if s0 == 0:
        b[:, 0:128] = MASK_NEG
        b[0, 0] = 0.0
        b[:, 128:384] = MASK_NEG
    else:
        b[:, 0:128] = interior[:, 256:384]
        b[:, 128:384] = interior[:, 128:384]
    b[:, 384:768] = interior
    return b.astype(BF16_NP)


def _make_in_maps(query_seq, keys_seq, values_seq, Wq, Wo):
    q = np.ascontiguousarray(np.asarray(query_seq, dtype=np.float32))
    k = np.ascontiguousarray(np.asarray(keys_seq, dtype=np.float32))
    v = np.ascontiguousarray(np.asarray(values_seq, dtype=np.float32))
    wq = np.asarray(Wq, dtype=np.float32)
    wo = np.asarray(Wo, dtype=np.float32)

    scale = np.float32(math.sqrt(float(HD)))
    wqT = np.ascontiguousarray(wq.T / scale).astype(BF16_NP)
    woT = np.ascontiguousarray(wo.T)

    in_maps = []
    for core in range(NCORES):
        b, ch = divmod(core, S // C)
        s0 = ch * C

        qcT = np.ascontiguousarray(q[b, s0:s0 + C, :].T).astype(BF16_NP)  # [MD, C]

        khalo = np.zeros((HALO, AD), np.float32)
        vhalo = np.zeros((HALO, AD), np.float32)
        lo = s0 - WIN
        off = max(0, -lo)
        khalo[off:] = k[b, lo + off:s0 + C, :]
        vhalo[off:] = v[b, lo + off:s0 + C, :]

        kT = np.ascontiguousarray(
            khalo.reshape(HALO, NH, HD).transpose(1, 2, 0)).astype(BF16_NP)

        vp = np.zeros((NKB, 128, VROW), BF16_NP)
        vh = vhalo.reshape(NKB, 128, NH, HD)
        for h in range(NH):
            vp[:, :, h * (HD + 1):h * (HD + 1) + HD] = vh[:, :, h, :].astype(BF16_NP)
            vp[:, :, h * (HD + 1) + HD] = 1.0

        in_maps.append({
            "qcT": qcT,
            "wqT": wqT,
            "woT": woT,
            "kT": kT,
            "vp": vp,
            "biasT": _build_bias(s0),
        })
    return in_maps


def _gather(results) -> np.ndarray:
    out = np.empty((B, S, MD), np.float32)
    for core in range(NCORES):
        b, ch = divmod(core, S // C)
        out[b, ch * C:(ch + 1) * C, :] = results[core]["out"]
    return out


def _run(in_maps, **kwargs):
    nc = _build_program()
    return run_bass_kernel_spmd(nc, in_maps, list(range(NCORES)), **kwargs)


def kernel(query_seq, keys_seq, values_seq, Wq, Wo, window=WIN, **_unused):
    assert int(window) == WIN, f"kernel hardcodes window={WIN}, got {window}"
    in_maps = _make_in_maps(query_seq, keys_seq, values_seq, Wq, Wo)
    res = _run(in_maps)
    return _gather(res.results)


def kernel_traced(query_seq, keys_seq, values_seq, Wq, Wo, window=WIN, **_unused):
    """Like kernel() but also returns BassKernelResults (profile/exec time)."""
    assert int(window) == WIN
    in_maps = _make_in_maps(query_seq, keys_seq, values_seq, Wq, Wo)
    res = _run(in_maps, trace=True)
    return _gather(res.results), res
